# revision 21
# baseline (speedup 1.0000x reference)
"""Cross-correlation layer kernel for Trainium2 (Bass/Tile), SPMD over 8 cores.

Problem: out[b, k, t] = sum_c x1[b, c, t] * x2p[b, c, t + 2D - k]
with x2p = zero-pad(x2, D) along time, D = 10, k in [0, 21).

Full shapes: x1, x2: [16, 512, 8192] fp32 -> out: [16, 21, 8192] fp32.

Sharding: pure data parallel over batch. Each of the 8 cores gets 2 batches
and computes its [2, 21, 8192] slice locally; host concatenates.

Per-core algorithm:
  Inputs are cast fp32->bf16 during the DMA load (SWDGE cast path); for each
  time block of 128 (t0) the PE accumulates over 4 channel chunks in fp32 PSUM:
      G[u, jj] = sum_c x1[c, t0+u] * x2p[c, t0+jj],  u in [0,128), jj in [0,148)
  The needed outputs are the 21 band diagonals  out[20-d, t0+u] = G[u, u+d].
  A per-partition skewed read is not expressible on-chip (compute-engine and
  DMA access patterns apply the same free offsets to every partition; the
  HWDGE partition step keeps only a 4-bit byte skew, verified on HW), so G
  blocks are staged into a wide SBUF tile and dumped per half-slab to a DRAM
  scratch, where the diagonal becomes a plain strided pattern: with row
  stride SW2, element (u, blk, d) sits at (SW2+1)*u + 148*blk + d, so a few
  long read runs per row (garbage between the 21-wide windows) cover all
  blocks' diagonals with only 128-256 descriptors per gather. A DVE copy
  packs the [128, (blk, 21)] strided columns, a PE transpose (identity
  matmul) flips to [(blk, d), u], and one DMA writes 512B-contiguous runs
  into out[b, k, :] (negative k-stride realizes k = 20 - d).

  The work is cut into per-batch time segments; the final batch ends with
  descending segment sizes (…, 2048, 1024, 512, 512) so the last segment's
  matmul+extract chain — which cannot overlap any remaining loads — is short.

  Measured on the 8 axon trn2 cores: ~219 us HW exec for the uniform-4096
  version; loads-only floor ~198 us; max rel err ~3.5e-3 vs fp32 reference.
"""

import numpy as np

import concourse.bass as bass
import concourse.mybir as mybir
import concourse.tile as tile
from concourse import bacc
from concourse.masks import make_identity

D = 10
K = 2 * D + 1  # 21 displacements

F32 = mybir.dt.float32
F32R = mybir.dt.float32r
BF16 = mybir.dt.bfloat16


def build_nc(B, C, T, segs, group, n_cores=8, mode="bf16", do_mm=True, do_extract=True):
    """Build the per-core Bass program for inputs [B, C, T] -> out [B, K, T].

    segs: per-batch list of segment-width lists (each sums to T, % 512 == 0).
    mode: "bf16" (SWDGE cast loads, bf16 matmul, N=148)
          "f32r" (HWDGE fp32 loads, fp32r matmul, N padded to 256)
    """
    assert C % 128 == 0 and len(segs) == B
    for bsegs in segs:
        assert sum(bsegs) == T
        for sg in bsegs:
            assert sg % 512 == 0
    NCC = C // 128  # channel chunks
    GW = 148  # G width: 128 + 2D
    f32r = mode == "f32r"
    in_dt = F32 if f32r else BF16
    # fp32r needs moving dim >= 256 for full rate; extra columns are junk
    MMW = 256 if f32r else GW
    stg_dt = BF16 if not f32r else F32  # staging/dump/gather dtype
    slab_max = max(max(bsegs) for bsegs in segs)
    x2w_of = lambda sg: sg + (128 if f32r else 2 * D)

    nc = bacc.Bacc("TRN2", target_bir_lowering=False, num_devices=n_cores, num_swdge_queues=2)
    x1 = nc.dram_tensor("x1", [B, C, T], F32, kind="ExternalInput")
    x2 = nc.dram_tensor("x2", [B, C, T], F32, kind="ExternalInput")
    out = nc.dram_tensor("out", [B, K, T], F32, kind="ExternalOutput")
    # DRAM scratch: all half-slab G dumps concatenated, row-major per region
    gdr = nc.dram_tensor("gscratch", [B * (T // 128) * 128 * GW], stg_dt)

    with tile.TileContext(nc) as tc:
        with (
            tc.tile_pool(name="x1p", bufs=2 * NCC) as x1p,
            tc.tile_pool(name="x2p", bufs=2 * NCC) as x2p,
            tc.tile_pool(name="gsb", bufs=3) as gsbp,
            tc.tile_pool(name="diag", bufs=3) as diagp,
            tc.tile_pool(name="outp", bufs=4) as outp,
            tc.tile_pool(name="const", bufs=1) as constp,
            tc.tile_pool(name="ps", bufs=6, space="PSUM") as psp,
            tc.tile_pool(name="pst", bufs=2, space="PSUM") as pstp,
        ):
            ident = constp.tile([128, 128], stg_dt)
            make_identity(nc, ident[:, :])

            gdr_off = 0  # running scratch offset (elements)
            for b in range(B):
                ts0 = 0
                for sg in segs[b]:
                    slab = sg
                    nblk_slab = slab // 128
                    HB = nblk_slab // 2  # blocks per half-slab dump
                    SW = nblk_slab * GW
                    SW2 = HB * GW
                    x2w = x2w_of(slab)
                    # ---- load input slabs (SWDGE casts fp32->bf16) ----
                    x1t = [
                        x1p.tile([128, slab_max], in_dt, name="x1s", tag="x1s")
                        for _ in range(NCC)
                    ]
                    x2t = [
                        x2p.tile([128, x2w_of(slab_max)], in_dt, name="x2s", tag="x2s")
                        for _ in range(NCC)
                    ]
                    ldeng = nc.sync if f32r else nc.gpsimd
                    lo = ts0 - D  # x2 tile covers [ts0 - D, ts0 - D + x2w)
                    lo_c = max(0, lo)
                    hi_c = min(T, lo + x2w)
                    for cc in range(NCC):
                        if lo_c > lo:
                            nc.vector.memset(x2t[cc][:, 0 : lo_c - lo], 0.0)
                        if hi_c < lo + x2w:
                            nc.vector.memset(x2t[cc][:, hi_c - lo : x2w], 0.0)

                    # the very last segment cannot hide its matmuls behind a
                    # following segment's loads, so load it in time-subwindows
                    # and interleave matmuls; other segments use 8 big DMAs
                    # (best SWDGE efficiency) with matmuls after
                    is_last = b == B - 1 and ts0 + slab == T
                    SUBW = 1024 if (is_last and slab % 1024 == 0) else slab
                    gsb = gsbp.tile([128, slab_max * GW // 128], stg_dt, name="gsb", tag="gsb")
                    IG = 4 if nblk_slab % 4 == 0 else 2

                    def emit_mm_group(g0):
                        gtiles = [
                            psp.tile([128, MMW], F32, name="gps", tag="gps")
                            for _ in range(IG)
                        ]
                        for cc in range(NCC):
                            for j in range(IG):
                                u0 = (g0 + j) * 128
                                lhs = x1t[cc][:, u0 : u0 + 128]
                                rhs = x2t[cc][:, u0 : u0 + MMW]
                                if f32r:
                                    lhs = lhs.bitcast(F32R)
                                    rhs = rhs.bitcast(F32R)
                                nc.tensor.matmul(
                                    gtiles[j][:, :],
                                    lhs,
                                    rhs,
                                    start=(cc == 0),
                                    stop=(cc == NCC - 1),
                                )
                        for j in range(IG):
                            blk = g0 + j
                            nc.vector.tensor_copy(
                                gsb[:, blk * GW : (blk + 1) * GW],
                                gtiles[j][:, 0:GW],
                            )

                    for sub in range(slab // SUBW):
                        xs0 = sub * SUBW
                        # x2 window ends 2D past the x1 window so this
                        # subwindow's matmuls have their full moving range
                        x2s0 = 0 if sub == 0 else xs0 + 2 * D
                        x2s1 = x2w if xs0 + SUBW == slab else xs0 + SUBW + 2 * D
                        for cc in range(NCC):
                            c0 = cc * 128
                            ldeng.dma_start(
                                x1t[cc][:, xs0 : xs0 + SUBW],
                                x1[b, c0 : c0 + 128, ts0 + xs0 : ts0 + xs0 + SUBW],
                            )
                            w0 = max(lo_c - lo, x2s0)
                            w1 = min(hi_c - lo, x2s1)
                            ldeng.dma_start(
                                x2t[cc][:, w0:w1],
                                x2[b, c0 : c0 + 128, lo + w0 : lo + w1],
                            )
                        if do_mm:
                            for g0 in range(xs0 // 128, (xs0 + SUBW) // 128, IG):
                                emit_mm_group(g0)

                    # half-slab dumps + gathers: long runs per u covering the
                    # diagonal windows (garbage between 21-wide windows);
                    # split into 2 runs/row when long enough to stay >=512B
                    dviews = []  # (dview, nblk_in_view, blk0_abs)
                    blk_base = ts0 // 128
                    for h in range(2 if do_extract else 0):
                        nc.sync.dma_start(
                            bass.AP(gdr, gdr_off, [[SW2, 128], [1, SW2]]),
                            gsb[:, h * SW2 : (h + 1) * SW2],
                        )
                        nrun = 2 if HB >= 8 else 1
                        HBr = HB // nrun
                        RW = GW * (HBr - 1) + K  # run width per u
                        dtile = diagp.tile(
                            [128, slab_max * GW // 256], stg_dt, name="dt", tag="diag"
                        )
                        for r in range(nrun):
                            src = bass.AP(
                                gdr,
                                gdr_off + r * HBr * GW,
                                [[SW2 + 1, 128], [1, RW]],
                            )
                            nc.scalar.dma_start(
                                dtile[:, r * RW : (r + 1) * RW], src
                            )
                            # dtile[u, r*RW + GW*bb + d] = G_bb[u, u+d]; the
                            # view spans HBr*GW cols but only cols < RW of the
                            # run are ever read (d < K <= GW)
                            dviews.append(
                                (
                                    dtile[:, r * RW : r * RW + HBr * GW]
                                    .rearrange("p (bb j) -> p bb j", j=GW),
                                    HBr,
                                    blk_base + (h * HB) + r * HBr,
                                )
                            )
                        gdr_off += 128 * SW2
                    # ---- per group: pack strided cols, transpose, store ----
                    for dview, nbv, blk0_abs in dviews:
                        gs = max(1, min(group, nbv))
                        for gl in range(nbv // gs):
                            GF = gs * K
                            # pack [128, (gs, K)] strided cols -> contiguous
                            pk = outp.tile([128, group * K], stg_dt, name="pk", tag="pk")
                            nc.vector.tensor_copy(
                                pk[:, 0:GF],
                                dview[:, gl * gs : (gl + 1) * gs, 0:K],
                            )
                            tps = pstp.tile([group * K, 128], stg_dt, name="tps", tag="tps")
                            nc.tensor.transpose(
                                tps[0:GF, :], pk[:, 0:GF], ident[:, :]
                            )
                            osb = outp.tile([group * K, 128], F32, name="osb", tag="osb")
                            nc.vector.tensor_copy(osb[0:GF, :], tps[0:GF, :])
                            # out[b, 20-d, blk*128 + u] ; iterate (blkd, d, u)
                            blk0 = blk0_abs + gl * gs
                            dst = bass.AP(
                                out,
                                (b * K + 2 * D) * T + blk0 * 128,
                                [[128, gs], [-T, K], [1, 128]],
                            )
                            nc.sync.dma_start(dst, osb[0:GF, :])
                    ts0 += slab

            if not do_extract:
                dummy = constp.tile([128, 16], F32, name="dummy")
                nc.vector.memset(dummy[:, :], 0.0)
                nc.sync.dma_start(
                    bass.AP(out, 0, [[16, 128], [1, 16]]), dummy[:, :]
                )

    nc.compile()
    return nc


_NC_CACHE = {}


def _get_nc(B, C, T, segs, group, n_cores, mode):
    key = (B, C, T, tuple(map(tuple, segs)), group, n_cores, mode)
    if key not in _NC_CACHE:
        _NC_CACHE[key] = build_nc(B, C, T, segs, group, n_cores=n_cores, mode=mode)
    return _NC_CACHE[key]


def run_sharded(x1, x2, segs=None, group=4, mode="bf16", trace=False, **spmd_kwargs):
    """Run the SPMD kernel on 8 cores over full inputs; returns (out, results)."""
    from concourse.bass_utils import run_bass_kernel_spmd

    n_cores = 8
    Bf, C, T = x1.shape
    assert Bf % n_cores == 0
    Bs = Bf // n_cores
    if segs is None:
        segs = [[4096, 4096]] * Bs
    nc = _get_nc(Bs, C, T, segs, group, n_cores, mode)
    in_maps = [
        {
            "x1": np.ascontiguousarray(x1[i * Bs : (i + 1) * Bs]),
            "x2": np.ascontiguousarray(x2[i * Bs : (i + 1) * Bs]),
        }
        for i in range(n_cores)
    ]
    res = run_bass_kernel_spmd(
        nc, in_maps, core_ids=list(range(n_cores)), trace=trace, **spmd_kwargs
    )
    out = np.concatenate([r["out"] for r in res.results], axis=0)
    return out, res


def kernel(x1, x2):
    x1 = np.asarray(x1, dtype=np.float32)
    x2 = np.asarray(x2, dtype=np.float32)
    out, _ = run_sharded(x1, x2)
    return out
